# revision 1
# baseline (speedup 1.0000x reference)
"""Trainium2 Bass kernel for ChannelSelfCorrelation (fp16 pipeline).

Reference computation (per sample, X = x[b] viewed as (C=1024, N=1024)):
    Q = Wq @ X + bq,  K = Wk @ X + bk          (1x1 convs, channel GEMMs)
    S = Q_r @ K_r^T  (Q_r = Q as (C, HW) matrix since C == H*W == 1024)
    A = softmax_rows(S)                        (1024 x 1024)
    O = A @ X,  Y = Wo @ O + bo
Sharding: data-parallel over batch B=32 across 8 cores (4 samples/core).

Device formulation (zero transposes, fp16 matmuls with f32 PSUM accum):
    QT[p, o] = sum_c X[c, p] WqT[c, o]   lhsT=X-slice, rhs=WqT   (pixel-major)
    KT[p, o] likewise
    S[n, m]  = sum_p QT[p, n] KT[p, m]
    A[n, m]  = exp(S - rowmax) / rowsum  (ACT exp + per-partition scale)
    Z[m, o]  = sum_n A[n, m] WoT[n, o]   (= (Wo @ A)^T)
    Y[o, k]  = sum_m Z[m, o] X[m, k]

fp16 (e5m10) operands keep rel_l2 ~3e-3 (validated off-device), while:
  - fp16 weight loads use FWL and hide fully under the matmul stream
    (f32r self-loads don't, costing ~11ns per matmul);
  - all DMA/SBUF footprints halve, so the three weight matrices stay
    resident in SBUF and sample-0 startup is shorter.
Matmul outputs stay 512 wide (one PSUM bank limit); two column-half
accumulation groups share each [128,1024] PSUM tile so PSUM->SBUF
evacuations run 1024 wide. Phases 1+2 run k-outer over 4 concurrent
pb-groups so sample-0 compute keeps pace with the x/weight DMA stream.
Warm-up: dummy matmuls from t~0 keep the PE HAM clock-gate at 8/8 so the
first real matmuls run at 2.4GHz instead of 1.2GHz.
"""
import sys
import types

sys.path.insert(0, "/opt/trn_rl_repo")

import antenv  # noqa: E402

if "antenv.axon_hooks" not in sys.modules:
    _m = types.ModuleType("antenv.axon_hooks")
    _m._hook = None

    def _set_hook(h):
        _m._hook = h

    def _get_hook():
        return _m._hook

    _m.set_axon_ntff_profile_hook = _set_hook
    _m.get_axon_ntff_profile_hook = _get_hook
    sys.modules["antenv.axon_hooks"] = _m
    antenv.axon_hooks = _m
    try:
        from trn_agent_boot.trn_boot import _ntff_profile_via_ctypes

        _set_hook(_ntff_profile_via_ctypes("/opt/axon/libaxon_pjrt.so"))
    except Exception:
        pass

from contextlib import ExitStack  # noqa: E402

import numpy as np  # noqa: E402

import concourse.bacc as bacc  # noqa: E402
import concourse.tile as tile  # noqa: E402
from concourse import mybir  # noqa: E402
from concourse.bass_utils import run_bass_kernel_spmd  # noqa: E402

F16 = mybir.dt.float16
F32 = mybir.dt.float32
AF = mybir.ActivationFunctionType

B, C, H, W = 32, 1024, 32, 32
HW = H * W
NCORES = 8
SPC = B // NCORES  # samples per core
P = 128
NT = C // P  # 8 k-tiles
HALF = C // 2  # 512: max matmul output width (one PSUM bank)
NWARM = 14  # HAM warm-up dummy matmuls: bridge engine-boot to first x/w DMA


def build_nc(with_bias):
    nc = bacc.Bacc(None, target_bir_lowering=False, debug=False)
    x = nc.dram_tensor("x", [SPC, C, HW], F16, kind="ExternalInput")
    wqT = nc.dram_tensor("wqT", [C, C], F16, kind="ExternalInput")
    wkT = nc.dram_tensor("wkT", [C, C], F16, kind="ExternalInput")
    woT = nc.dram_tensor("woT", [C, C], F16, kind="ExternalInput")
    if with_bias:
        bq = nc.dram_tensor("bq", [C], F16, kind="ExternalInput")
        bk = nc.dram_tensor("bk", [C], F16, kind="ExternalInput")
        bo = nc.dram_tensor("bo", [C], F32, kind="ExternalInput")
        onesd = nc.dram_tensor("onesd", [P], F16, kind="ExternalInput")
    y = nc.dram_tensor("y", [SPC, C, HW], F32, kind="ExternalOutput")

    with tile.TileContext(nc) as tc, ExitStack() as ctx:
        xp = ctx.enter_context(tc.tile_pool(name="xp", bufs=2))
        wp = ctx.enter_context(tc.tile_pool(name="wp", bufs=1))
        qp = ctx.enter_context(tc.tile_pool(name="qp", bufs=1))
        kp = ctx.enter_context(tc.tile_pool(name="kp", bufs=1))
        apool = ctx.enter_context(tc.tile_pool(name="apool", bufs=1))
        zp = ctx.enter_context(tc.tile_pool(name="zp", bufs=1))
        yst = ctx.enter_context(tc.tile_pool(name="yst", bufs=3))
        st = ctx.enter_context(tc.tile_pool(name="st", bufs=24))
        psp = ctx.enter_context(tc.tile_pool(name="psp", bufs=4, space="PSUM"))

        # --- PE warm-up: keep the HAM clock-gate open while DMA ramps ---
        sc = wp.tile([P, 256], F16, name="warm_src")
        nc.vector.memset(sc, 0.0)
        for i in range(NWARM):
            psw = psp.tile([P, C], F32, tag="mm", name=f"warm{i}")
            nc.tensor.matmul(psw[:, 0:256], sc[:, 0:P], sc[:],
                             start=True, stop=True)

        # --- Resident weights + sample-0 x, in consumption order ---
        wq_sb = wp.tile([P, NT, C], F16, name="wq_sb")
        wk_sb = wp.tile([P, NT, C], F16, name="wk_sb")
        wo_sb = wp.tile([P, NT, C], F16, name="wo_sb")
        wq_r = wqT.rearrange("(t p) o -> p t o", p=P)
        wk_r = wkT.rearrange("(t p) o -> p t o", p=P)
        wo_r = woT.rearrange("(t p) o -> p t o", p=P)

        xt = xp.tile([P, NT, HW], F16, tag="x", name="x0")
        xsrc = x[0].rearrange("(t p) n -> p t n", p=P)
        for k in range(NT):
            nc.sync.dma_start(out=xt[:, k, :], in_=xsrc[:, k, :])
            # Second queue (ACT-issued) so x and wq stream concurrently
            # through more DMA engines during the ramp-up window.
            nc.scalar.dma_start(out=wq_sb[:, k, :], in_=wq_r[:, k, :])
        if with_bias:
            cst = ctx.enter_context(tc.tile_pool(name="cst", bufs=1))
            ones = cst.tile([1, P], F16, name="ones")
            nc.sync.dma_start(out=ones, in_=onesd.rearrange("(a p) -> a p", a=1))
            bq_sb = cst.tile([1, C], F16, name="bq_sb")
            nc.sync.dma_start(out=bq_sb, in_=bq.rearrange("(a c) -> a c", a=1))
            bk_sb = cst.tile([1, C], F16, name="bk_sb")
            nc.sync.dma_start(out=bk_sb, in_=bk.rearrange("(a c) -> a c", a=1))
            bo_sb = cst.tile([P, NT], F32, name="bo_sb")
            nc.sync.dma_start(out=bo_sb, in_=bo.rearrange("(t p) -> p t", p=P))
        for k in range(NT):
            nc.sync.dma_start(out=wk_sb[:, k, :], in_=wk_r[:, k, :])
        for k in range(NT):
            nc.sync.dma_start(out=wo_sb[:, k, :], in_=wo_r[:, k, :])

        for s in range(SPC):
            # ---- Phases 1+2: QT / KT (pixel-major Q and K) ----
            # k-outer over 4 concurrent pb-groups: sample-0 matmuls consume
            # (x-k, w-k) pairs as they land instead of waiting for all 8.
            qt = qp.tile([P, NT, C], F16, tag="qt", name=f"qt{s}")
            kt = kp.tile([P, NT, C], F16, tag="kt", name=f"kt{s}")
            for w_sb, bslot, dst, evict in (
                (wq_sb, 0, qt, "act"),
                (wk_sb, 1, kt, "dve"),
            ):
                for half4 in range(2):
                    pbs = range(4 * half4, 4 * (half4 + 1))
                    pss = {
                        pb: psp.tile([P, C], F32, tag="mm",
                                     name=f"psqk{s}_{bslot}_{pb}")
                        for pb in pbs
                    }
                    for k in range(NT):
                        for pb in pbs:
                            for ch in range(2):
                                cs = slice(HALF * ch, HALF * (ch + 1))
                                nc.tensor.matmul(
                                    pss[pb][:, cs],
                                    xt[:, k, P * pb:P * (pb + 1)],
                                    w_sb[:, k, cs],
                                    start=(k == 0),
                                    stop=(not with_bias and k == NT - 1),
                                )
                    for pb in pbs:
                        if with_bias:
                            b_sb = bq_sb if bslot == 0 else bk_sb
                            for ch in range(2):
                                cs = slice(HALF * ch, HALF * (ch + 1))
                                nc.tensor.matmul(
                                    pss[pb][:, cs], ones[:, :], b_sb[:, cs],
                                    start=False, stop=True,
                                )
                        if evict == "act":
                            nc.scalar.activation(dst[:, pb, :], pss[pb][:],
                                                 AF.Copy)
                        else:
                            nc.vector.tensor_copy(dst[:, pb, :], pss[pb][:])

            # ---- Phase 3: S + softmax -> A (row-major, n x m) ----
            # Prefetch next sample's x while the PE is busy with S.
            if s + 1 < SPC:
                xt_next = xp.tile([P, NT, HW], F16, tag="x", name=f"x{s + 1}")
                xsrc_n = x[s + 1].rearrange("(t p) n -> p t n", p=P)
                for k in range(NT):
                    nc.sync.dma_start(out=xt_next[:, k, :], in_=xsrc_n[:, k, :])
            at = apool.tile([P, NT, C], F16, tag="a", name=f"a{s}")
            for nb in range(NT):
                ps = psp.tile([P, C], F32, tag="mm", name=f"pss{s}_{nb}")
                for ch in range(2):
                    cs = slice(HALF * ch, HALF * (ch + 1))
                    for k in range(NT):
                        nc.tensor.matmul(
                            ps[:, cs],
                            qt[:, k, P * nb:P * (nb + 1)],
                            kt[:, k, cs],
                            start=(k == 0),
                            stop=(k == NT - 1),
                        )
                negmax = st.tile([P, 1], F32, tag="stat", name=f"ngm{s}_{nb}")
                nc.vector.tensor_reduce(
                    negmax, ps[:], axis=mybir.AxisListType.X,
                    op=mybir.AluOpType.max, negate=True,
                )
                rs = st.tile([P, 1], F32, tag="stat", name=f"rs{s}_{nb}")
                nc.scalar.activation(
                    at[:, nb, :], ps[:], AF.Exp, bias=negmax, accum_out=rs,
                )
                rcp = st.tile([P, 1], F32, tag="stat", name=f"rcp{s}_{nb}")
                nc.vector.reciprocal(rcp[:], rs[:])
                nc.scalar.activation(
                    at[:, nb, :], at[:, nb, :], AF.Identity, scale=rcp[:],
                )

            # ---- Phase 4: Z = A^T @ WoT  (m x o) ----
            zt = zp.tile([P, NT, C], F16, tag="z", name=f"z{s}")
            for mb in range(NT):
                ps = psp.tile([P, C], F32, tag="mm", name=f"psz{s}_{mb}")
                for ch in range(2):
                    cs = slice(HALF * ch, HALF * (ch + 1))
                    for k in range(NT):
                        nc.tensor.matmul(
                            ps[:, cs],
                            at[:, k, P * mb:P * (mb + 1)],
                            wo_sb[:, k, cs],
                            start=(k == 0),
                            stop=(k == NT - 1),
                        )
                nc.vector.tensor_copy(zt[:, mb, :], ps[:])

            # ---- Phase 5: Y = Z^T @ X + bo  (o x k = channels x pixels) ----
            for ob in range(NT):
                last = (s == SPC - 1 and ob == NT - 1)
                if not last:
                    ps = psp.tile([P, C], F32, tag="mm", name=f"psy{s}_{ob}")
                    for ch in range(2):
                        cs = slice(HALF * ch, HALF * (ch + 1))
                        for k in range(NT):
                            nc.tensor.matmul(
                                ps[:, cs],
                                zt[:, k, P * ob:P * (ob + 1)],
                                xt[:, k, cs],
                                start=(k == 0),
                                stop=(k == NT - 1),
                            )
                    ysb = yst.tile([P, C], F32, tag="y", name=f"y{s}_{ob}")
                    if with_bias:
                        nc.scalar.activation(
                            ysb[:], ps[:], AF.Identity, bias=bo_sb[:, ob:ob + 1],
                        )
                    else:
                        nc.scalar.activation(ysb[:], ps[:], AF.Copy)
                    # Last sample: alternate store queues so the final store
                    # doesn't queue behind this backlog (in-order queues).
                    eng = nc.scalar if (s == SPC - 1 and ob % 2 == 1) else nc.sync
                    eng.dma_start(
                        out=y[s, P * ob:P * (ob + 1), :], in_=ysb[:],
                    )
                    continue
                # Final output block: two independent 512-wide groups so the
                # first half's evac+store pipelines behind the second half's
                # matmuls; full-width rows keep the store DMA descriptor-
                # efficient (4KB+2KB bursts, multi-engine spread).
                for gi in range(2):
                    cs = slice(HALF * gi, HALF * (gi + 1))
                    ps = psp.tile([P, C], F32, tag="mm",
                                  name=f"psy{s}_{ob}_{gi}")
                    for k in range(NT):
                        nc.tensor.matmul(
                            ps[:, cs],
                            zt[:, k, P * ob:P * (ob + 1)],
                            xt[:, k, cs],
                            start=(k == 0),
                            stop=(k == NT - 1),
                        )
                    ysb = yst.tile([P, C], F32, tag="y", name=f"y{s}_{ob}_{gi}")
                    if with_bias:
                        nc.scalar.activation(
                            ysb[:, cs], ps[:, cs], AF.Identity,
                            bias=bo_sb[:, ob:ob + 1],
                        )
                    elif gi == 1:
                        # Final evac split across ACT and DVE in parallel to
                        # shorten the post-matmul tail.
                        q4 = slice(HALF, HALF + 256)
                        q5 = slice(HALF + 256, C)
                        nc.scalar.activation(ysb[:, q4], ps[:, q4], AF.Copy)
                        nc.vector.tensor_copy(ysb[:, q5], ps[:, q5])
                    else:
                        nc.scalar.activation(ysb[:, cs], ps[:, cs], AF.Copy)
                    if gi == 0:
                        nc.sync.dma_start(
                            out=y[s, P * ob:P * (ob + 1), cs], in_=ysb[:, cs],
                        )
                    else:
                        # The very last store: one DMA entry runs on a single
                        # engine (~47GB/s), so split it into two partition-half
                        # entries on two different queues to engage two
                        # engines concurrently.
                        nc.sync.dma_start(
                            out=y[s, P * ob:P * ob + 64, cs],
                            in_=ysb[0:64, cs],
                        )
                        nc.scalar.dma_start(
                            out=y[s, P * ob + 64:P * (ob + 1), cs],
                            in_=ysb[64:P, cs],
                        )
            if s + 1 < SPC:
                xt = xt_next

    nc.compile()
    return nc


_NC_CACHE = {}


def _get_nc(with_bias):
    if with_bias not in _NC_CACHE:
        _NC_CACHE[with_bias] = build_nc(with_bias)
    return _NC_CACHE[with_bias]


def run(x, Wq, bq, Wk, bk, Wo, bo, trace=False):
    """Shard, execute on 8 cores, gather. Returns (y_full, BassKernelResults)."""
    x = np.asarray(x, dtype=np.float32).reshape(B, C, HW).astype(np.float16)
    wqT = np.ascontiguousarray(np.asarray(Wq, dtype=np.float32).T).astype(np.float16)
    wkT = np.ascontiguousarray(np.asarray(Wk, dtype=np.float32).T).astype(np.float16)
    woT = np.ascontiguousarray(np.asarray(Wo, dtype=np.float32).T).astype(np.float16)
    bq = np.asarray(bq, dtype=np.float32)
    bk = np.asarray(bk, dtype=np.float32)
    bo = np.ascontiguousarray(np.asarray(bo, dtype=np.float32))

    with_bias = bool(bq.any() or bk.any() or bo.any())
    nc = _get_nc(with_bias)
    in_maps = []
    for i in range(NCORES):
        m = {
            "x": np.ascontiguousarray(x[SPC * i:SPC * (i + 1)]),
            "wqT": wqT, "wkT": wkT, "woT": woT,
        }
        if with_bias:
            m.update({"bq": bq.astype(np.float16), "bk": bk.astype(np.float16),
                      "bo": bo, "onesd": np.ones(P, np.float16)})
        in_maps.append(m)
    res = run_bass_kernel_spmd(
        nc, in_maps, core_ids=list(range(NCORES)), trace=trace,
    )
    y = np.concatenate([res.results[i]["y"] for i in range(NCORES)], axis=0)
    return y.reshape(B, C, H, W), res


def kernel(x, Wq, bq, Wk, bk, Wo, bo):
    y, _ = run(x, Wq, bq, Wk, bk, Wo, bo, trace=False)
    return y

